# revision 1
# baseline (speedup 1.0000x reference)
"""GNN message-passing kernel (max+mean aggregation -> linear -> log_softmax)
for Trainium2, 8 NeuronCores, dst-node sharding.

Strategy:
- Shard destination nodes: core c owns global nodes [c*12500, (c+1)*12500),
  padded to 12544 = 98*128 local slots.
- Host sorts each core's nodes by in-degree and builds a SHARED degree
  template T[p] = max over cores of the p-th sorted degree, so one SPMD
  program serves all 8 cores; per-core index data pads missing slots with a
  neutral row.
- Neighbor features are gathered on-device with indirect DMA (int32 row
  indices) from xg = concat([zeros row], x + SHIFT). The shift makes the
  zero pad row neutral for max; pads add exactly 0 to sums; the shift is
  cancelled exactly by folding -SHIFT * rowsum(W) into the bias.
- Gathered slot tiles are PE-transposed to [feat, slot] layout, then DVE
  tensor_reduce (max and add) over degree-equal segments accumulates
  agg_max / agg_sum in SBUF [128 feat, 12544 nodes].
- Projection per 128-node chunk: PSUM matmuls Wl_max@agg_max, Wl_mean@agg_sum
  (scaled by 1/deg post-transpose), (Wr_max+Wr_mean)@x, bias, then fused
  log_softmax, DMA out.
"""

import os
import sys

os.environ.setdefault("NEURON_RT_RESET_CORES", "1")
if "/opt/trn_rl_repo" not in sys.path:
    sys.path.insert(0, "/opt/trn_rl_repo")

import numpy as np

import concourse.mybir as mybir
from concourse import bacc, bass, tile
from concourse.masks import make_identity

N_NODES = 100000
D = 128
NCLS = 40
NCORES = 8
NPC = 12500
NPAD = 12544  # 98 * 128
NPROJ = NPAD // 128  # 98
CHUNK = 1536  # gather-chunk slots
IPC = CHUNK // 128  # indirect instrs per chunk
SHIFT = 12.0

last_exec_time_ns = None


def _plan(dst):
    """Per-core degree sort + shared template + chunk/piece layout."""
    core = dst // NPC
    degs = np.zeros((NCORES, NPAD), np.int64)
    orders = np.zeros((NCORES, NPAD), np.int64)
    sdeg = np.zeros((NCORES, NPAD), np.int64)
    for c in range(NCORES):
        dloc = np.bincount(dst[core == c] - c * NPC, minlength=NPC)
        degs[c, :NPC] = dloc
        o = np.argsort(degs[c], kind="stable")
        orders[c] = o
        sdeg[c] = degs[c][o]
    T = sdeg.max(axis=0)

    chunks = []
    p = 0
    while p < NPAD:
        cap = CHUNK
        q = p
        while q < NPAD and T[q] <= cap:
            cap -= T[q]
            q += 1
        chunks.append((p, q))
        p = q

    pieces = []  # per chunk: list of (slot_off, col0, nb, d)
    node_slot_start = np.zeros(NPAD, np.int64)
    for ci, (a, b) in enumerate(chunks):
        node_slot_start[a:b] = ci * CHUNK + np.concatenate(
            [[0], np.cumsum(T[a:b])[:-1]]
        )
        pl = []
        off = 0
        i = a
        while i < b:
            j = i
            while j < b and T[j] == T[i]:
                j += 1
            if T[i] > 0:
                pl.append((int(off), int(i), int(j - i), int(T[i])))
            off += (j - i) * int(T[i])
            i = j
        pieces.append(pl)
    return degs, orders, sdeg, T, chunks, pieces, node_slot_start


def _core_idx(src_c, dstloc_c, order, sdeg_c, node_slot_start, total_slots):
    """int32 slot->xg-row index array for one core (0 = neutral pad row)."""
    pos = np.empty(NPAD, np.int64)
    pos[order] = np.arange(NPAD)
    key = pos[dstloc_c]
    eorder = np.argsort(key, kind="stable")
    s_sorted = src_c[eorder]
    first = np.concatenate([[0], np.cumsum(sdeg_c)[:-1]])
    rank = np.arange(len(s_sorted)) - np.repeat(first, sdeg_c)
    positions = np.repeat(node_slot_start, sdeg_c) + rank
    idx = np.zeros(total_slots, np.int64)
    idx[positions] = s_sorted + 1
    return idx


def _build_program(nchunks, pieces, chunk_ranges):
    nc = bacc.Bacc()
    f32 = mybir.dt.float32
    ncols = nchunks * IPC

    # projection chunk pc is ready once gather chunk ci finalizes all acc
    # cols < (pc+1)*128; emit it right after that chunk's reduces
    proj_after = [[] for _ in range(nchunks)]
    pc = 0
    for ci, (a, b) in enumerate(chunk_ranges):
        while pc < NPROJ and (pc + 1) * 128 <= b:
            proj_after[ci].append(pc)
            pc += 1
    while pc < NPROJ:
        proj_after[-1].append(pc)
        pc += 1

    xg_in = nc.declare_dram_parameter("xg", [N_NODES + 1, D], f32, isOutput=False)
    idx_in = nc.declare_dram_parameter("idx", [128, ncols], mybir.dt.int32,
                                       isOutput=False)
    xT_in = nc.declare_dram_parameter("xT", [D, NPAD], f32, isOutput=False)
    invd_in = nc.declare_dram_parameter("invd", [128, NPROJ], f32, isOutput=False)
    fix_in = nc.declare_dram_parameter("fix", [128, NPROJ, NCLS], f32,
                                       isOutput=False)
    wlmaxT_in = nc.declare_dram_parameter("wlmaxT", [D, NCLS], f32, isOutput=False)
    wlmeanT_in = nc.declare_dram_parameter("wlmeanT", [D, NCLS], f32,
                                           isOutput=False)
    wrcT_in = nc.declare_dram_parameter("wrcT", [D, NCLS], f32, isOutput=False)
    o_out = nc.declare_dram_parameter("out", [NPAD, NCLS], f32, isOutput=True)

    with tile.TileContext(nc) as tc:
        with tc.tile_pool(name="persist", bufs=1) as pers:
            idx_t = pers.tile([128, ncols], mybir.dt.int32)
            invd_t = pers.tile([128, NPROJ], f32)
            fix_t = pers.tile([128, NPROJ, NCLS], f32)
            wlmaxT_t = pers.tile([D, NCLS], f32)
            wlmeanT_t = pers.tile([D, NCLS], f32)
            wrcT_t = pers.tile([D, NCLS], f32)
            ident_t = pers.tile([128, 128], f32)
            acc_max = pers.tile([128, NPAD], f32)
            acc_sum = pers.tile([128, NPAD], f32)

            nc.sync.dma_start(out=idx_t[:, :], in_=idx_in[:, :])
            nc.sync.dma_start(out=invd_t[:, :], in_=invd_in[:, :])
            nc.sync.dma_start(out=fix_t[:, :, :], in_=fix_in[:, :, :])
            nc.sync.dma_start(out=wlmaxT_t[:, :], in_=wlmaxT_in[:, :])
            nc.sync.dma_start(out=wlmeanT_t[:, :], in_=wlmeanT_in[:, :])
            nc.sync.dma_start(out=wrcT_t[:, :], in_=wrcT_in[:, :])
            make_identity(nc, ident_t)
            nc.vector.memset(acc_max[:, :], 0.0)
            nc.vector.memset(acc_sum[:, :], 0.0)

            with tc.tile_pool(name="gath", bufs=4) as gpool, \
                 tc.tile_pool(name="gpsum", bufs=2, space="PSUM") as ppool, \
                 tc.tile_pool(name="proj", bufs=2) as proj, \
                 tc.tile_pool(name="ppsum", bufs=2, space="PSUM") as prps:

                def emit_proj(pc):
                    c0 = pc * 128
                    xT_t = proj.tile([D, 128], f32, name="xTc")
                    nc.sync.dma_start(out=xT_t[:, :], in_=xT_in[:, c0:c0 + 128])

                    # one PSUM bank: [:40, 0:128]=mean mm, [:40,128:256]=
                    # max+root mm, [:,256:296]/[:,296:336]=transposes
                    ps = prps.tile([128, 336], f32, name="ps")
                    nc.tensor.matmul(ps[:NCLS, 0:128], wlmeanT_t[:, :],
                                     acc_sum[:, c0:c0 + 128],
                                     start=True, stop=True)
                    nc.tensor.matmul(ps[:NCLS, 128:256], wlmaxT_t[:, :],
                                     acc_max[:, c0:c0 + 128],
                                     start=True, stop=False)
                    nc.tensor.matmul(ps[:NCLS, 128:256], wrcT_t[:, :],
                                     xT_t[:, :], start=False, stop=True)

                    sA = proj.tile([NCLS, 128], f32, name="sA")
                    sB = proj.tile([NCLS, 128], f32, name="sB")
                    nc.scalar.copy(sA[:, :], ps[:NCLS, 0:128])
                    nc.scalar.copy(sB[:, :], ps[:NCLS, 128:256])
                    nc.tensor.transpose(ps[:, 256:296], sA[:, :],
                                        ident_t[:NCLS, :NCLS])
                    nc.tensor.transpose(ps[:, 296:336], sB[:, :],
                                        ident_t[:NCLS, :NCLS])

                    z = proj.tile([128, NCLS], f32, name="z")
                    nc.vector.tensor_scalar(
                        out=z[:, :], in0=ps[:, 256:296],
                        scalar1=invd_t[:, pc:pc + 1], scalar2=None,
                        op0=mybir.AluOpType.mult,
                    )
                    nc.vector.tensor_tensor(z[:, :], z[:, :], ps[:, 296:336],
                                            mybir.AluOpType.add)
                    nc.vector.tensor_tensor(z[:, :], z[:, :], fix_t[:, pc, :],
                                            mybir.AluOpType.add)

                    m = proj.tile([128, 1], f32, name="m")
                    nc.vector.tensor_reduce(out=m[:, :], in_=z[:, :],
                                            axis=mybir.AxisListType.X,
                                            op=mybir.AluOpType.max)
                    negm = proj.tile([128, 1], f32, name="negm")
                    nc.vector.tensor_scalar(
                        out=negm[:, :], in0=m[:, :], scalar1=-1.0,
                        scalar2=None, op0=mybir.AluOpType.mult,
                    )
                    e = proj.tile([128, NCLS], f32, name="e")
                    se = proj.tile([128, 1], f32, name="se")
                    nc.scalar.activation(
                        e[:, :], z[:, :], mybir.ActivationFunctionType.Exp,
                        bias=negm[:, :1], scale=1.0, accum_out=se[:, :1],
                    )
                    ls = proj.tile([128, 1], f32, name="ls")
                    nc.scalar.activation(ls[:, :], se[:, :],
                                         mybir.ActivationFunctionType.Ln)
                    nc.vector.tensor_tensor(ls[:, :], ls[:, :], m[:, :],
                                            mybir.AluOpType.add)
                    ot = proj.tile([128, NCLS], f32, name="ot")
                    nc.vector.tensor_scalar(
                        out=ot[:, :], in0=z[:, :], scalar1=ls[:, :1],
                        scalar2=None, op0=mybir.AluOpType.subtract,
                    )
                    nc.sync.dma_start(out=o_out[c0:c0 + 128, :], in_=ot[:, :])

                for ci in range(nchunks):
                    g = gpool.tile([128, IPC, D], f32, name="g")
                    for k in range(IPC):
                        col = ci * IPC + k
                        nc.gpsimd.indirect_dma_start(
                            out=g[:, k, :],
                            out_offset=None,
                            in_=xg_in[:, :],
                            in_offset=bass.IndirectOffsetOnAxis(
                                ap=idx_t[:, col:col + 1], axis=0
                            ),
                        )
                    pt = ppool.tile([128, CHUNK], f32, name="pt")
                    for b in range(IPC):
                        nc.tensor.transpose(
                            pt[:, b * 128:(b + 1) * 128], g[:, b, :], ident_t
                        )
                    for (off, col0, nb, dd) in pieces[ci]:
                        seg = pt[:, off:off + nb * dd].rearrange(
                            "p (nb d) -> p nb d", d=dd
                        )
                        nc.vector.tensor_reduce(
                            out=acc_max[:, col0:col0 + nb], in_=seg,
                            axis=mybir.AxisListType.X, op=mybir.AluOpType.max,
                        )
                        nc.vector.tensor_reduce(
                            out=acc_sum[:, col0:col0 + nb], in_=seg,
                            axis=mybir.AxisListType.X, op=mybir.AluOpType.add,
                        )
                    for pc in proj_after[ci]:
                        emit_proj(pc)
    return nc


def kernel(**inputs):
    global last_exec_time_ns
    x = np.asarray(inputs["x"], dtype=np.float32)
    ei = np.asarray(inputs["edge_index"]).astype(np.int64)
    Wl_max = np.asarray(inputs["Wl_max"], dtype=np.float32)
    Wr_max = np.asarray(inputs["Wr_max"], dtype=np.float32)
    b_max = np.asarray(inputs["b_max"], dtype=np.float32)
    Wl_mean = np.asarray(inputs["Wl_mean"], dtype=np.float32)
    Wr_mean = np.asarray(inputs["Wr_mean"], dtype=np.float32)
    b_mean = np.asarray(inputs["b_mean"], dtype=np.float32)

    src, dst = ei[0], ei[1]
    degs, orders, sdeg, T, chunks, pieces, nss = _plan(dst)
    nchunks = len(chunks)
    total_slots = nchunks * CHUNK
    ncols = total_slots // 128

    xg = np.zeros((N_NODES + 1, D), np.float32)
    xg[1:] = x + SHIFT

    rs = SHIFT * (Wl_max.sum(axis=1) + Wl_mean.sum(axis=1))  # [40]
    bias_eff = b_max + b_mean - rs
    wlmaxT = np.ascontiguousarray(Wl_max.T)
    wlmeanT = np.ascontiguousarray(Wl_mean.T)
    wrcT = np.ascontiguousarray((Wr_max + Wr_mean).T)

    core = dst // NPC
    in_maps = []
    for c in range(NCORES):
        msk = core == c
        idx = _core_idx(src[msk], dst[msk] - c * NPC, orders[c], sdeg[c],
                        nss, total_slots)
        idx_t = np.ascontiguousarray(
            idx.reshape(ncols, 128).T).astype(np.int32)

        ids = orders[c]
        real = ids < NPC
        xo = np.zeros((NPAD, D), np.float32)
        xo[real] = x[c * NPC + ids[real]]
        xT = np.ascontiguousarray(xo.T)

        invd = (1.0 / np.maximum(sdeg[c], 1)).astype(np.float32)
        invd_t = np.ascontiguousarray(invd.reshape(NPROJ, 128).T)

        fix = np.tile(bias_eff, (NPAD, 1)).astype(np.float32)
        fix[sdeg[c] == 0] += rs
        fix_t = np.ascontiguousarray(
            fix.reshape(NPROJ, 128, NCLS).transpose(1, 0, 2))

        in_maps.append({
            "xg": xg, "idx": idx_t, "xT": xT, "invd": invd_t, "fix": fix_t,
            "wlmaxT": wlmaxT, "wlmeanT": wlmeanT, "wrcT": wrcT,
        })

    nc = _build_program(nchunks, pieces, chunks)
    nc.compile()

    from concourse.bass_utils import run_bass_kernel_spmd
    res = run_bass_kernel_spmd(nc, in_maps, list(range(NCORES)))
    if os.environ.get("GNN_TRACE", "0") == "1":
        # separate single-core traced run: tracing the 8-core run crashes
        # the exec unit; core 0's time is representative (identical program)
        tr = run_bass_kernel_spmd(nc, in_maps[:1], [0], trace=True)
        last_exec_time_ns = tr.exec_time_ns

    out = np.zeros((N_NODES, NCLS), np.float32)
    for c in range(NCORES):
        o = np.asarray(res.results[c]["out"])
        ids = orders[c]
        real = ids < NPC
        out[c * NPC + ids[real]] = o[real]
    return out



# revision 9
# speedup vs baseline: 1.1802x; 1.1802x over previous
"""GNN message-passing kernel (SAGEConv max+mean -> linear -> log_softmax)
for Trainium2, 8 NeuronCores, dst-node sharding.

v3 design:
- Gather via InstDMAGatherAnt (gpsimd.dma_gather): ONE instruction gathers
  3072 rows (2 chunks), amortizing the ~1us SWDGE fixed cost ~24x vs the
  per-128-row indirect DMA -> Pool engine drops from ~1.7ms to ~140us.
  * int16 indices: each core's sources are deduplicated per "bank" of 22
    chunks (~28.5k unique rows < 32k signed-int16 range) into a compacted
    per-core HBM array xb; gathers use per-bank base offsets.
  * HW quirk (queue 0): the Q7 ucode reads index k from partition 16+k%16,
    while CoreSim's interp reads partition k%16 -> indices are written at
    BOTH partition ranges so sim and HW both see them.
  * single_packet=False: coalescing 3072 descriptors into one DMA packet
    crashes the exec unit; per-descriptor packets work.
- fp16 pipeline: halves gather bytes; PE transposes at 1 cyc/row into fp16
  PSUM; projection matmuls all-fp16.
- Pad handling without SHIFT: bank row 0 = -16.0 (loses every max against
  real fp16 x ~ N(0,1)); mean-path pads are zeroed at the source because
  the segment-sum weights (seglib) are 0 on pad rows; deg==0 nodes get an
  exact +16*rowsum(Wl_max) correction folded into the per-node bias `fix`.
- Mean path on PE: per 128-slot tile, one matmul
  psum_win[feat, node] += g_tile[slot, feat].T @ seg[slot, node], where
  seg holds 1/deg at real slot rows -> PSUM accumulates the mean directly
  in windows of 128 nodes aligned with projection chunks. DVE only does
  the segmented max reduce.
"""

import os
import sys

os.environ.setdefault("NEURON_RT_RESET_CORES", "1")
if "/opt/trn_rl_repo" not in sys.path:
    sys.path.insert(0, "/opt/trn_rl_repo")

import numpy as np

import concourse.mybir as mybir
from concourse import bacc, bass, tile
from concourse.masks import make_identity

N_NODES = 100000
D = 128
NCLS = 40
NCORES = 8
NPC = 12500
NPAD = 12544  # 98 * 128
NPROJ = NPAD // 128  # 98
CHUNK = 1536  # slots per processing chunk
IPC = CHUNK // 128  # 128-slot tiles per chunk
GB = 2  # chunks per gather instruction
CPB = 22  # chunks per bank (int16 index range)
PAD_VAL = -16.0

last_exec_time_ns = None


def _plan(dst):
    """Per-core degree sort + shared template + chunk/tile/window layout."""
    core = dst // NPC
    degs = np.zeros((NCORES, NPAD), np.int64)
    orders = np.zeros((NCORES, NPAD), np.int64)
    sdeg = np.zeros((NCORES, NPAD), np.int64)
    for c in range(NCORES):
        dloc = np.bincount(dst[core == c] - c * NPC, minlength=NPC)
        degs[c, :NPC] = dloc
        o = np.argsort(degs[c], kind="stable")
        orders[c] = o
        sdeg[c] = degs[c][o]
    T = sdeg.max(axis=0)

    chunks = []
    p = 0
    while p < NPAD:
        cap = CHUNK
        q = p
        while q < NPAD and T[q] <= cap:
            cap -= T[q]
            q += 1
        chunks.append((p, q))
        p = q
    if len(chunks) % GB:  # keep gathers uniform (2 whole chunks each)
        chunks.append((NPAD, NPAD))

    nchunks = len(chunks)
    total_slots = nchunks * CHUNK

    node_slot_start = np.zeros(NPAD, np.int64)
    slot2p = np.full(total_slots, -1, np.int64)  # -1 = chunk-tail pad
    for ci, (a, b) in enumerate(chunks):
        node_slot_start[a:b] = ci * CHUNK + np.concatenate(
            [[0], np.cumsum(T[a:b])[:-1]]
        )
        n = int(T[a:b].sum())
        if n:
            slot2p[ci * CHUNK: ci * CHUNK + n] = np.repeat(
                np.arange(a, b), T[a:b]
            )

    # pieces per chunk for the DVE max reduce: runs of equal T
    pieces = []
    for ci, (a, b) in enumerate(chunks):
        pl = []
        off = 0
        i = a
        while i < b:
            j = i
            while j < b and T[j] == T[i]:
                j += 1
            if T[i] > 0:
                pl.append((int(off), int(i), int(j - i), int(T[i])))
            off += (j - i) * int(T[i])
            i = j
        pieces.append(pl)

    # library columns: runs of equal (tile, p) over slots
    ntiles = total_slots // 128
    tid = np.arange(total_slots) // 128
    newrun = np.ones(total_slots, bool)
    newrun[1:] = (slot2p[1:] != slot2p[:-1]) | (tid[1:] != tid[:-1])
    real = slot2p >= 0
    runid = np.full(total_slots, -1, np.int64)
    runid[real] = np.cumsum(newrun & real)[real] - 1
    nlib = int(runid.max()) + 1 if real.any() else 0

    run_starts = np.where(newrun & real)[0]
    run_tile = tid[run_starts]
    run_p = slot2p[run_starts]
    run_gcol = runid[run_starts]
    segmms = []  # (t, w, g0, ncols, c0)
    for t in range(ntiles):
        sel = np.where(run_tile == t)[0]
        if len(sel) == 0:
            continue
        ps = run_p[sel]
        gs = run_gcol[sel]
        w0 = ps // 128
        k = 0
        while k < len(sel):
            m = k
            while m < len(sel) and w0[m] == w0[k]:
                m += 1
            segmms.append(
                (t, int(w0[k]), int(gs[k]), int(m - k), int(ps[k] % 128))
            )
            k = m
    return (degs, orders, sdeg, T, chunks, pieces, node_slot_start,
            slot2p, runid, nlib, segmms)


def _core_slot_src(src_c, dstloc_c, order, sdeg_c, node_slot_start,
                   total_slots):
    """Per-slot global source id for one core (-1 = pad)."""
    pos = np.empty(NPAD, np.int64)
    pos[order] = np.arange(NPAD)
    key = pos[dstloc_c]
    eorder = np.argsort(key, kind="stable")
    s_sorted = src_c[eorder]
    first = np.concatenate([[0], np.cumsum(sdeg_c)[:-1]])
    rank = np.arange(len(s_sorted)) - np.repeat(first, sdeg_c)
    positions = np.repeat(node_slot_start, sdeg_c) + rank
    slot_src = np.full(total_slots, -1, np.int64)
    slot_src[positions] = s_sorted
    return slot_src


def _build_program(nchunks, pieces, chunks, nlib, segmms, ncols16,
                   nbrows, bank_off):
    nc = bacc.Bacc()
    f32 = mybir.dt.float32
    f16 = mybir.dt.float16
    i16 = mybir.dt.int16
    ngath = nchunks // GB
    nidx = GB * CHUNK  # 3072 per gather
    icol = nidx // 16  # 192 idx columns per gather

    win_last_chunk = [0] * NPROJ
    segmm_by_chunk = [[] for _ in range(nchunks)]
    for (t, w, g0, ncol, c0) in segmms:
        ci = t // IPC
        segmm_by_chunk[ci].append((t, w, g0, ncol, c0))
        win_last_chunk[w] = max(win_last_chunk[w], ci)
    proj_after = [[] for _ in range(nchunks)]
    for w in range(NPROJ):
        proj_after[win_last_chunk[w]].append(w)
    win_mm_count = [0] * NPROJ
    for (t, w, g0, ncol, c0) in segmms:
        win_mm_count[w] += 1

    xb_in = nc.declare_dram_parameter("xb", [nbrows, D], f16, isOutput=False)
    idx_in = nc.declare_dram_parameter("idx", [128, ncols16], i16,
                                       isOutput=False)
    lib_in = nc.declare_dram_parameter("lib", [128, max(nlib, 1)], f16,
                                       isOutput=False)
    xT_in = nc.declare_dram_parameter("xT", [D, NPAD], f16, isOutput=False)
    fix_in = nc.declare_dram_parameter("fix", [128, NPROJ, NCLS], f32,
                                       isOutput=False)
    wlmaxT_in = nc.declare_dram_parameter("wlmaxT", [D, NCLS], f16,
                                          isOutput=False)
    wlmeanT_in = nc.declare_dram_parameter("wlmeanT", [D, NCLS], f16,
                                           isOutput=False)
    wrcT_in = nc.declare_dram_parameter("wrcT", [D, NCLS], f16,
                                        isOutput=False)
    o_out = nc.declare_dram_parameter("out", [NPAD, NCLS], f32, isOutput=True)

    with tile.TileContext(nc) as tc:
        with tc.tile_pool(name="persist", bufs=1) as pers:
            idx_t = pers.tile([128, ncols16], i16)
            lib_t = pers.tile([128, max(nlib, 1)], f16)
            xT_t = pers.tile([D, NPAD], f16)
            fix_t = pers.tile([128, NPROJ, NCLS], f32)
            wlmaxT_t = pers.tile([D, NCLS], f16)
            wlmeanT_t = pers.tile([D, NCLS], f16)
            wrcT_t = pers.tile([D, NCLS], f16)
            ident_t = pers.tile([128, 128], f16)
            zero_t = pers.tile([128, 128], f16)
            acc_max = pers.tile([128, NPAD], f16)

            nc.sync.dma_start(out=idx_t[:, :], in_=idx_in[:, :])
            nc.sync.dma_start(out=lib_t[:, :], in_=lib_in[:, :])
            nc.sync.dma_start(out=xT_t[:, :], in_=xT_in[:, :])
            nc.sync.dma_start(out=fix_t[:, :, :], in_=fix_in[:, :, :])
            nc.sync.dma_start(out=wlmaxT_t[:, :], in_=wlmaxT_in[:, :])
            nc.sync.dma_start(out=wlmeanT_t[:, :], in_=wlmeanT_in[:, :])
            nc.sync.dma_start(out=wrcT_t[:, :], in_=wrcT_in[:, :])
            make_identity(nc, ident_t)
            nc.vector.memset(zero_t[:, :], 0.0)
            nc.vector.memset(acc_max[:, :], 0.0)

            with tc.tile_pool(name="gath", bufs=3) as gpool, \
                 tc.tile_pool(name="tpsum", bufs=2, space="PSUM") as tpp, \
                 tc.tile_pool(name="wpsum", bufs=2, space="PSUM") as wpp, \
                 tc.tile_pool(name="proj", bufs=2) as proj, \
                 tc.tile_pool(name="ppsum", bufs=1, space="PSUM") as prps:

                win_tiles = {}

                def emit_proj(w):
                    c0 = w * 128
                    pw = win_tiles.pop(w)
                    zsum = proj.tile([128, 128], f16, name="zsum")
                    nc.scalar.copy(zsum[:, :], pw[:, :])

                    ps1 = prps.tile([NCLS, 128], f32, name="ps1")
                    nc.tensor.matmul(ps1[:, :], wlmeanT_t[:, :], zsum[:, :],
                                     start=True, stop=False)
                    nc.tensor.matmul(ps1[:, :], wlmaxT_t[:, :],
                                     acc_max[:, c0:c0 + 128],
                                     start=False, stop=False)
                    nc.tensor.matmul(ps1[:, :], wrcT_t[:, :],
                                     xT_t[:, c0:c0 + 128],
                                     start=False, stop=True)

                    sA = proj.tile([NCLS, 128], f16, name="sA")
                    nc.scalar.copy(sA[:, :], ps1[:, :])
                    ps2 = prps.tile([128, NCLS], f16, name="ps2")
                    nc.tensor.transpose(ps2[:, :], sA[:, :],
                                        ident_t[:NCLS, :NCLS])

                    zf = proj.tile([128, NCLS], f32, name="zf")
                    nc.vector.tensor_tensor(zf[:, :], ps2[:, :],
                                            fix_t[:, w, :],
                                            mybir.AluOpType.add)
                    m = proj.tile([128, 1], f32, name="m")
                    nc.vector.tensor_reduce(out=m[:, :], in_=zf[:, :],
                                            axis=mybir.AxisListType.X,
                                            op=mybir.AluOpType.max)
                    negm = proj.tile([128, 1], f32, name="negm")
                    nc.vector.tensor_scalar(
                        out=negm[:, :], in0=m[:, :], scalar1=-1.0,
                        scalar2=None, op0=mybir.AluOpType.mult,
                    )
                    e = proj.tile([128, NCLS], f32, name="e")
                    se = proj.tile([128, 1], f32, name="se")
                    nc.scalar.activation(
                        e[:, :], zf[:, :], mybir.ActivationFunctionType.Exp,
                        bias=negm[:, :1], scale=1.0, accum_out=se[:, :1],
                    )
                    ls = proj.tile([128, 1], f32, name="ls")
                    nc.scalar.activation(ls[:, :], se[:, :],
                                         mybir.ActivationFunctionType.Ln)
                    nc.vector.tensor_tensor(ls[:, :], ls[:, :], m[:, :],
                                            mybir.AluOpType.add)
                    ot = proj.tile([128, NCLS], f32, name="ot")
                    nc.vector.tensor_scalar(
                        out=ot[:, :], in0=zf[:, :], scalar1=ls[:, :1],
                        scalar2=None, op0=mybir.AluOpType.subtract,
                    )
                    nc.sync.dma_start(out=o_out[c0:c0 + 128, :], in_=ot[:, :])

                for gi in range(ngath):
                    ca = gi * GB
                    g = gpool.tile([128, GB * IPC, D], f16, name="g")
                    nc.gpsimd.dma_gather(
                        out_ap=g[:, :, :],
                        in_ap=xb_in[bank_off[gi]:, :],
                        idxs_ap=idx_t[:, gi * icol:(gi + 1) * icol],
                        num_idxs=nidx,
                        num_idxs_reg=nidx,
                        elem_size=D,
                        single_packet=False,
                    )
                    for ci in range(ca, ca + GB):
                        k0 = (ci - ca) * IPC
                        pt = tpp.tile([128, CHUNK], f16, name="pt")
                        for b in range(IPC):
                            nc.tensor.transpose(
                                pt[:, b * 128:(b + 1) * 128],
                                g[:, k0 + b, :], ident_t
                            )
                        for (off, col0, nb, dd) in pieces[ci]:
                            seg = pt[:, off:off + nb * dd].rearrange(
                                "p (nb d) -> p nb d", d=dd
                            )
                            nc.vector.tensor_reduce(
                                out=acc_max[:, col0:col0 + nb], in_=seg,
                                axis=mybir.AxisListType.X,
                                op=mybir.AluOpType.max,
                            )
                        for (t, w, g0, ncol, c0) in segmm_by_chunk[ci]:
                            if w not in win_tiles:
                                pw = wpp.tile([128, 128], f32, name="win")
                                win_tiles[w] = pw
                                nc.tensor.matmul(
                                    pw[:, :], zero_t[:, :], zero_t[:, :],
                                    start=True, stop=False,
                                    skip_group_check=True,
                                )
                            win_mm_count[w] -= 1
                            nc.tensor.matmul(
                                win_tiles[w][:, c0:c0 + ncol],
                                g[:, k0 + (t % IPC), :],
                                lib_t[:, g0:g0 + ncol],
                                start=False, stop=(win_mm_count[w] == 0),
                                skip_group_check=True,
                            )
                        for w in proj_after[ci]:
                            if w not in win_tiles:
                                pw = wpp.tile([128, 128], f32, name="win")
                                win_tiles[w] = pw
                                nc.tensor.matmul(
                                    pw[:, :], zero_t[:, :], zero_t[:, :],
                                    start=True, stop=True,
                                    skip_group_check=True,
                                )
                            emit_proj(w)
    return nc


def _host_prep(inputs):
    """Plan + per-core input arrays. Returns (build_args, final_maps,
    (orders,))."""
    x = np.asarray(inputs["x"], dtype=np.float32)
    ei = np.asarray(inputs["edge_index"]).astype(np.int64)
    Wl_max = np.asarray(inputs["Wl_max"], dtype=np.float32)
    Wr_max = np.asarray(inputs["Wr_max"], dtype=np.float32)
    b_max = np.asarray(inputs["b_max"], dtype=np.float32)
    Wl_mean = np.asarray(inputs["Wl_mean"], dtype=np.float32)
    Wr_mean = np.asarray(inputs["Wr_mean"], dtype=np.float32)
    b_mean = np.asarray(inputs["b_mean"], dtype=np.float32)

    src, dst = ei[0], ei[1]
    (degs, orders, sdeg, T, chunks, pieces, nss, slot2p, runid, nlib,
     segmms) = _plan(dst)
    nchunks = len(chunks)
    total_slots = nchunks * CHUNK
    ngath = nchunks // GB
    nbanks = (nchunks + CPB - 1) // CPB

    x16 = x.astype(np.float16)
    rsmax = Wl_max.sum(axis=1)
    bias_eff = b_max + b_mean
    wlmaxT = np.ascontiguousarray(Wl_max.T.astype(np.float16))
    wlmeanT = np.ascontiguousarray(Wl_mean.T.astype(np.float16))
    wrcT = np.ascontiguousarray((Wr_max + Wr_mean).T.astype(np.float16))

    s_all = np.arange(total_slots)
    rank_all = np.where(slot2p >= 0, s_all - nss[np.maximum(slot2p, 0)], -1)

    # bank of each gather instruction + slot->bank
    gath_bank = np.arange(ngath) * GB // CPB
    slot_bank = (s_all // CHUNK) // CPB

    core = dst // NPC
    in_maps = []
    bank_off = None
    for c in range(NCORES):
        msk = core == c
        slot_src = _core_slot_src(src[msk], dst[msk] - c * NPC, orders[c],
                                  sdeg[c], nss, total_slots)

        # per-bank dedup + compact int16 indices
        idx16 = np.zeros(total_slots, np.int16)
        xb_parts = []
        offs = []
        row_base = 0
        pad_row = np.full((1, D), PAD_VAL, np.float16)
        for b in range(nbanks):
            sel = (slot_bank == b) & (slot_src >= 0)
            uniq, inv = np.unique(slot_src[sel], return_inverse=True)
            assert len(uniq) + 1 <= 32767, f"bank {b} overflow: {len(uniq)}"
            idx16[sel] = (inv + 1).astype(np.int16)
            offs.append(row_base)
            xb_parts.append(pad_row)
            xb_parts.append(x16[uniq])
            row_base += 1 + len(uniq)
        xb = np.ascontiguousarray(np.concatenate(xb_parts, axis=0))
        if bank_off is None:
            bank_off = [offs[gath_bank[gi]] for gi in range(ngath)]
        else:
            # program is shared across cores: per-gather bank offsets must
            # match. They do by construction only if bank row counts match,
            # so pad banks to the template max AFTER collecting all cores.
            bank_off_c = [offs[gath_bank[gi]] for gi in range(ngath)]
            assert bank_off_c is not None
        # NOTE: bank offsets differ per core; handled below by padding xb
        # to uniform bank sizes. Store raw pieces for later fixup.
        in_maps.append({
            "slot_src": slot_src, "idx16": idx16, "xb_parts": None,
            "offs": offs, "xb": xb,
        })

    # Uniform bank offsets across cores: pad each core's banks to the max
    # bank size so the shared program's base offsets are identical.
    max_bank_rows = [0] * nbanks
    for c in range(NCORES):
        offs = in_maps[c]["offs"]
        ends = offs[1:] + [in_maps[c]["xb"].shape[0]]
        for b in range(nbanks):
            max_bank_rows[b] = max(max_bank_rows[b], ends[b] - offs[b])
    uoffs = np.concatenate([[0], np.cumsum(max_bank_rows)])[:-1]
    nbrows = int(sum(max_bank_rows))
    bank_off = [int(uoffs[gath_bank[gi]]) for gi in range(ngath)]

    ncols16 = ngath * (GB * CHUNK // 16)
    final_maps = []
    for c in range(NCORES):
        offs = in_maps[c]["offs"]
        xb_c = in_maps[c]["xb"]
        ends = offs[1:] + [xb_c.shape[0]]
        xb_u = np.zeros((nbrows, D), np.float16)
        for b in range(nbanks):
            n = ends[b] - offs[b]
            xb_u[uoffs[b]:uoffs[b] + n] = xb_c[offs[b]:ends[b]]
        idx16 = in_maps[c]["idx16"]
        # idx tile: gather gi consumes [128, 192] block; index k of that
        # gather lives at partition k%16 (sim) AND 16+k%16 (HW queue 0).
        idx_arr = np.zeros((128, ncols16), np.int16)
        k = np.arange(total_slots)
        gi = k // (GB * CHUNK)
        kk = k % (GB * CHUNK)
        colbase = gi * (GB * CHUNK // 16)
        idx_arr[kk % 16, colbase + kk // 16] = idx16
        idx_arr[16 + (kk % 16), colbase + kk // 16] = idx16

        slot_src = in_maps[c]["slot_src"]
        invd = (1.0 / np.maximum(sdeg[c], 1)).astype(np.float32)
        w_slot = np.zeros(total_slots, np.float32)
        rmask = slot2p >= 0
        w_slot[rmask] = np.where(
            rank_all[rmask] < sdeg[c][slot2p[rmask]],
            invd[slot2p[rmask]], 0.0,
        )
        lib = np.zeros((128, max(nlib, 1)), np.float16)
        lib[s_all[rmask] % 128, runid[rmask]] = w_slot[rmask].astype(
            np.float16)

        ids = orders[c]
        real = ids < NPC
        xo = np.zeros((NPAD, D), np.float16)
        xo[real] = x16[c * NPC + ids[real]]
        xT = np.ascontiguousarray(xo.T)

        fix = np.tile(bias_eff, (NPAD, 1)).astype(np.float32)
        fix[(T > 0) & (sdeg[c] == 0)] -= PAD_VAL * rsmax
        fix_t = np.ascontiguousarray(
            fix.reshape(NPROJ, 128, NCLS).transpose(1, 0, 2))

        final_maps.append({
            "xb": xb_u, "idx": idx_arr, "lib": lib, "xT": xT, "fix": fix_t,
            "wlmaxT": wlmaxT, "wlmeanT": wlmeanT, "wrcT": wrcT,
        })

    build_args = (nchunks, pieces, chunks, nlib, segmms, ncols16,
                  nbrows, bank_off)
    return build_args, final_maps, orders


def kernel(**inputs):
    global last_exec_time_ns
    build_args, final_maps, orders = _host_prep(inputs)
    nc = _build_program(*build_args)
    nc.compile()

    from concourse.bass_utils import run_bass_kernel_spmd
    res = run_bass_kernel_spmd(nc, final_maps, list(range(NCORES)))
    if os.environ.get("GNN_TRACE", "0") == "1":
        # separate single-core traced run: tracing the 8-core run crashes
        # the exec unit; core 0's time is representative (identical program)
        tr = run_bass_kernel_spmd(nc, final_maps[:1], [0], trace=True)
        last_exec_time_ns = tr.exec_time_ns

    out = np.zeros((N_NODES, NCLS), np.float32)
    for c in range(NCORES):
        o = np.asarray(res.results[c]["out"])
        ids = orders[c]
        real = ids < NPC
        out[c * NPC + ids[real]] = o[real]
    return out
